# revision 11
# baseline (speedup 1.0000x reference)
"""LorentzTransformer Trainium2 kernel (v3).

Full inputs in, full output out. Sharding: 8 cores = 2 batches x 4 head
groups (4 heads / 256 channels each). Host pre-transposes x and the weight
shards so every on-chip matmul has its contraction dim on partitions.

Per-core pipeline (fp16 PE datapath, fp32 PSUM accumulation):
  - big batched input DMAs split over the three DMA-capable queues
    (sync: xT, scalar: wq/wk/wv, gpsimd: consts+wo) so transfers overlap;
    projections are emitted k-outer so compute starts while x streams in
  - Q/K proj keep each weight tile stationary for both 512-col q chunks;
    all four dense projections are emitted back-to-back so the PE never
    waits on the lorentz DVE/ACT chain (HAM stays warm)
  - Qeff = Q * (1/scale - 2*alpha/scale*sf*m); norm sums via two 2-col PE
    matmuls (separate PSUM tiles so recip/mul inputs share partition base
    0 - multi-input DVE ops require equal input partition bases), then a
    fused (add,mult) scalar_tensor_tensor applies the factor in place
  - scoresT[k,q]: 2 heads row-packed into one [128,2,512] 2-bank PSUM
    tile -> ONE exp per k-tile covering both heads; causal via block
    skipping + triangular 0/1 mask multiplied on the (idle) GpSimd
  - V' carries 64 replicated ones columns so the AV matmul emits the
    softmax denominator replicated across partitions 64:128; normalize =
    shift-copy + reciprocal + two PSUM-direct muls per (t,qc) group
  - partial out = A @ Wo_shard.T interleaved between attention q-chunks;
    fp16 partials DMA'd out, host sums the 4 head-group partials
"""

import numpy as np

from concourse import bacc
import concourse.tile as tile
import concourse.mybir as mybir
from concourse.alu_op_type import AluOpType
from concourse.bass_utils import run_bass_kernel_spmd

B, L, D, H = 2, 1024, 1024, 16
DH = D // H  # 64
ALPHA = 0.25
SCALE = float(np.sqrt(DH))  # 8.0
HPC = 4          # heads per core
DPC = HPC * DH   # 256 channels per core
N_CORES = 8
P = 128
NK = D // P      # 8 contraction tiles
NQC = L // 512   # 2 q chunks of 512
NKT = L // P     # 8 k tiles of 128

FP = mybir.dt.float32
FPC = mybir.dt.float16
NPC = np.float16
AF = mybir.ActivationFunctionType


def _build_program():
    nc = bacc.Bacc("TRN2", target_bir_lowering=False)

    xT = nc.dram_tensor("xT", [D, L], FPC, kind="ExternalInput")
    wqT = nc.dram_tensor("wqT", [D, DPC], FPC, kind="ExternalInput")
    wkT = nc.dram_tensor("wkT", [D, DPC], FPC, kind="ExternalInput")
    wvT = nc.dram_tensor("wvT", [D, DPC], FPC, kind="ExternalInput")
    woT = nc.dram_tensor("woT", [DPC, D], FPC, kind="ExternalInput")
    normblk = nc.dram_tensor("normblk", [P, 2, 4], FPC, kind="ExternalInput")
    sprime = nc.dram_tensor("sprime", [2, 2, P], FPC, kind="ExternalInput")
    maskT = nc.dram_tensor("maskT", [P, 1, P], FPC, kind="ExternalInput")
    out = nc.dram_tensor("out", [L, D], FPC, kind="ExternalOutput")

    with tile.TileContext(nc) as tc:
        with (
            tc.tile_pool(name="persist", bufs=1) as persist,
            tc.tile_pool(name="work", bufs=2) as work,
            tc.tile_pool(name="expp", bufs=6) as expp,
            tc.tile_pool(name="sm", bufs=4) as smp,
            tc.tile_pool(name="sfp", bufs=2) as sfp,
            tc.tile_pool(name="rcp", bufs=4) as rcp,
            tc.tile_pool(name="ost", bufs=4) as ost,
            tc.tile_pool(name="psS", bufs=2, space="PSUM") as psS,
        ):
            # ---- batched input DMAs, split across the 3 DMA queues ----
            # one HWDGE ring in need-order: the ring drains FIFO, so the
            # first-needed tensors get the full HBM bandwidth instead of
            # round-robin sharing with later ones
            xT_sb = persist.tile([P, NK, L], FPC, tag="xT")
            xT_r = xT.rearrange("(o p) l -> p o l", p=P)
            wq_sb = persist.tile([P, NK, DPC], FPC, tag="wq")
            wq_r = wqT.rearrange("(o p) n -> p o n", p=P)
            nc.sync.dma_start(wq_sb[:, 0 : NK // 2], wq_r[:, 0 : NK // 2])
            nc.sync.dma_start(wq_sb[:, NK // 2 : NK], wq_r[:, NK // 2 : NK])
            for k in range(NK):
                nc.sync.dma_start(xT_sb[:, k], xT_r[:, k])
            wk_sb = persist.tile([P, NK, DPC], FPC, tag="wk")
            nc.sync.dma_start(wk_sb[:], wkT.rearrange("(o p) n -> p o n", p=P))
            wv_sb = persist.tile([P, NK, DPC], FPC, tag="wv")
            nc.sync.dma_start(wv_sb[:], wvT.rearrange("(o p) n -> p o n", p=P))
            wo_sb = persist.tile([P, DPC // P, D], FPC, tag="wo")
            nc.sync.dma_start(wo_sb[:], woT.rearrange("(o p) n -> p o n", p=P))
            nb_sb = persist.tile([P, 2, 4], FPC, tag="nb")
            nc.gpsimd.dma_start(nb_sb[:], normblk[:])
            sp_sb = persist.tile([2, 2, P], FPC, tag="sp")
            nc.gpsimd.dma_start(sp_sb[:], sprime[:])
            mk_sb = persist.tile([P, 1, P], FPC, tag="mk")
            nc.gpsimd.dma_start(mk_sb[:], maskT[:])

            # warm the sqrt activation table while inputs stream in (after
            # the DMA issues so the ~2.7us table load doesn't delay them)
            sqd = smp.tile([1, 8], FP, tag="sqd")
            nc.vector.memset(sqd[:], 1.0)
            sqd2 = smp.tile([1, 8], FP, tag="sqd2")
            nc.scalar.activation(sqd2[:], sqd[:], AF.Sqrt)

            qT_sb = [persist.tile([P, L], FPC, tag=f"qT{t}", name=f"qT{t}") for t in range(2)]
            kT_sb = [persist.tile([P, L], FPC, tag=f"kT{t}", name=f"kT{t}") for t in range(2)]
            # V' with 64 replicated ones columns per (ktile, head) -> the AV
            # matmul emits the softmax denominator on partitions 64:128
            v_sb = persist.tile([P, NKT, HPC, P], FPC, tag="v")
            ones64 = persist.tile([P, 1, 1, DH], FPC, tag="ones64")
            nc.vector.memset(ones64[:], 1.0)
            nc.vector.tensor_copy(
                v_sb[:, :, :, DH:P],
                ones64[:].to_broadcast([P, NKT, HPC, DH]),
            )

            aT_sb = [
                [
                    persist.tile([P, 512], FPC, tag=f"aT{t}_{qc}", name=f"aT{t}_{qc}")
                    for qc in range(NQC)
                ]
                for t in range(2)
            ]

            # ---- Q/K projection: weight tile stationary for both q chunks ----
            def proj(w_sb, dst, t):
                pss = [ps1.tile([P, 512], FP, tag="ps1", name=f"proj{qc}") for qc in range(NQC)]
                for k in range(NK):
                    for qc in range(NQC):
                        nc.tensor.matmul(
                            pss[qc][:],
                            w_sb[:, k, t * P : (t + 1) * P],
                            xT_sb[:, k, qc * 512 : (qc + 1) * 512],
                            start=(k == 0),
                            stop=(k == NK - 1),
                        )
                for qc in range(NQC):
                    nc.vector.tensor_copy(dst[t][:, qc * 512 : (qc + 1) * 512], pss[qc][:])

            sf_t = [None, None]

            def lorentz(t):
                # QeffT = QT * (0.125 - 0.0625*sf*m), sf = |Q|/|Qt| per (head,q)
                sq = work.tile([P, L], FPC, tag="sq")
                nc.vector.tensor_mul(sq[:], qT_sb[t][:], qT_sb[t][:])
                nn_p = psS.tile([P, 2, 512], FP, tag="psS", name="nn_p")
                nn_q = psS.tile([P, 2, 512], FP, tag="psS", name="nn_q")
                for qc in range(NQC):
                    nc.tensor.matmul(
                        nn_p[0:2, qc, :],
                        nb_sb[:, t, 0:2],
                        sq[:, qc * 512 : (qc + 1) * 512],
                        start=True,
                        stop=True,
                    )
                    nc.tensor.matmul(
                        nn_q[0:2, qc, :],
                        nb_sb[:, t, 2:4],
                        sq[:, qc * 512 : (qc + 1) * 512],
                        start=True,
                        stop=True,
                    )
                rr = smp.tile([2, 2, 512], FP, tag="rr")
                nc.vector.reciprocal_approx_fast(rr[:], nn_q[0:2, :, :])
                rat = smp.tile([2, 2, 512], FP, tag="rat")
                nc.vector.tensor_mul(rat[:], nn_p[0:2, :, :], rr[:])
                sf = sfp.tile([2, 2, 512], FPC, tag="sf")
                nc.scalar.activation(sf[:], rat[:], AF.Sqrt)
                sf_t[t] = sf
                for qc in range(NQC):
                    gps = ps1.tile([P, 512], FP, tag="ps1", name="gps")
                    nc.tensor.matmul(
                        gps[:],
                        sp_sb[:, t, :],
                        sf[0:2, qc, :],
                        start=True,
                        stop=True,
                    )
                    # qT = (gps + 1/scale) * qT fused on the DVE
                    nc.vector.scalar_tensor_tensor(
                        qT_sb[t][:, qc * 512 : (qc + 1) * 512],
                        gps[:],
                        1.0 / SCALE,
                        qT_sb[t][:, qc * 512 : (qc + 1) * 512],
                        AluOpType.add,
                        AluOpType.mult,
                    )

            # dense projections back-to-back keep the PE warm; the lorentz
            # DVE/ACT chains overlap the K projection matmuls.  ps1 is scoped
            # to this phase so its PSUM banks are free for attention.
            with tc.tile_pool(name="ps1", bufs=4, space="PSUM") as ps1:
                for t in range(2):
                    proj(wq_sb, qT_sb, t)
                for t in range(2):
                    proj(wk_sb, kT_sb, t)
                    lorentz(t)

                # switch the ACT table set to exp while K/V projections run;
                # reading sf_t[1] forces this after the last sqrt
                exd = smp.tile([1, 8], FPC, tag="exd")
                nc.scalar.activation(exd[:], sf_t[1][0:1, 0, 0:8], AF.Exp)

                # ---- V natural layout: out[l, dv], packed into V' ----
                for lt in range(NKT):
                    ps = ps1.tile([P, 512], FP, tag="ps1", name="vproj")
                    for k in range(NK):
                        nc.tensor.matmul(
                            ps[:, :DPC],
                            xT_sb[:, k, lt * P : (lt + 1) * P],
                            wv_sb[:, k, :],
                            start=(k == 0),
                            stop=(k == NK - 1),
                        )
                    nc.vector.tensor_copy(
                        v_sb[:, lt, :, :DH],
                        ps[:, :DPC].rearrange("p (h d) -> p h d", h=HPC),
                    )

            # ---- attention: 2 heads row-packed, one exp per k-tile ----
            def attn_group(t, qc):
                av = psV.tile([P, 2, 512], FP, tag="psV", name="av")
                nkt = 4 * qc + 4  # causal: k tiles 0..4qc+3

                def av_mm(kt, ex, off):
                    for hl in range(2):
                        nc.tensor.matmul(
                            av[:, hl, off:512],
                            v_sb[:, kt, 2 * t + hl, :],
                            ex[:, hl, off:512],
                            start=(kt == 0),
                            stop=(kt == nkt - 1),
                        )

                pend = None  # (kt, ex, off) whose AV is deferred one step
                for kt in range(nkt):
                    off = max(0, (kt - 4 * qc) * P)  # first visible q col
                    sc = psS.tile([P, 2, 512], FP, tag="psS", name="sc")
                    for hl in range(2):
                        base = hl * DH
                        nc.tensor.matmul(
                            sc[:, hl, off:512],
                            kT_sb[t][base : base + DH, kt * P : (kt + 1) * P],
                            qT_sb[t][
                                base : base + DH,
                                qc * 512 + off : (qc + 1) * 512,
                            ],
                            start=True,
                            stop=True,
                            tile_position=(base, 0),
                        )
                    # the deferred AV sits *behind* this kt's score matmuls in
                    # the PE queue, so an exp-gated AV never stalls the scores
                    # that feed the ACT engine
                    if pend is not None:
                        av_mm(*pend)
                    ex = expp.tile([P, 2, 512], FPC, tag="ex", name="ex")
                    nc.scalar.activation(ex[:, :, off:512], sc[:, :, off:512], AF.Exp)
                    j = kt - 4 * qc
                    if j >= 0:  # diagonal block gets the triangular mask
                        nc.gpsimd.tensor_mul(
                            ex[:, :, j * P : (j + 1) * P],
                            ex[:, :, j * P : (j + 1) * P],
                            mk_sb[:].to_broadcast([P, 2, P]),
                        )
                    pend = (kt, ex, off)
                av_mm(*pend)
                # normalize: denominator sits replicated on partitions 64:128;
                # shift-copy to base 0 (multi-input DVE ops need equal bases)
                den = rcp.tile([DH, 2, 512], FP, tag="den")
                nc.vector.tensor_copy(den[:], av[DH:P, :, :])
                rc = rcp.tile([DH, 2, 512], FP, tag="rc")
                nc.vector.reciprocal_approx_fast(rc[:], den[:])
                for hl in range(2):
                    nc.vector.tensor_mul(
                        aT_sb[t][qc][hl * DH : (hl + 1) * DH, :],
                        av[0:DH, hl, :],
                        rc[:, hl, :],
                    )

            def wo_tile(lt, evac="v"):
                qc = lt // 4
                oc = ost.tile([P, 2, 512], FPC, tag="oc")
                ps = psV.tile([P, 2, 512], FP, tag="psV", name="wops")
                for jc in range(NQC):
                    for t2 in range(2):
                        nc.tensor.matmul(
                            ps[:, jc, :],
                            aT_sb[t2][qc][:, (lt % 4) * P : (lt % 4 + 1) * P],
                            wo_sb[:, t2, jc * 512 : (jc + 1) * 512],
                            start=(t2 == 0),
                            stop=(t2 == 1),
                        )
                    if evac == "v":
                        nc.vector.tensor_copy(oc[:, jc, :], ps[:, jc, :])
                    else:
                        nc.scalar.copy(oc[:, jc, :], ps[:, jc, :])
                eng = nc.sync if lt % 2 == 0 else nc.scalar
                eng.dma_start(
                    out[lt * P : (lt + 1) * P, :], oc[:].rearrange("p a b -> p (a b)")
                )

            # psV holds two attention accumulators (or Wo tiles) at a time so
            # consecutive (t,qc) groups overlap across the normalize tail
            with tc.tile_pool(name="psV", bufs=2, space="PSUM") as psV:
                for t in range(2):
                    attn_group(t, 0)
                for lt in range(2):
                    wo_tile(lt, evac="v")
                for t in range(2):
                    attn_group(t, 1)
                for lt in range(2, NKT):
                    wo_tile(lt, evac="s")

    nc.compile()
    return nc


_NC = None


def _host_inputs(x, Wq, Wk, Wv, Wo, timelike_mask):
    m_full = np.asarray(timelike_mask).astype(np.float32)
    mt = np.tril(np.ones((P, P), dtype=np.float32)).T.copy()  # maskT[k,q]=1 iff k<=q
    in_maps = []
    for c in range(N_CORES):
        b, g = divmod(c, HPC)
        sl = slice(g * DPC, (g + 1) * DPC)
        m = m_full[sl]  # [256]
        nb = np.zeros((P, 2, 4), dtype=np.float32)
        sp = np.zeros((2, 2, P), dtype=np.float32)
        for t in range(2):
            m_t = m[t * P : (t + 1) * P]
            nb[0:DH, t, 0] = 1.0
            nb[DH:P, t, 1] = 1.0
            nb[0:DH, t, 2] = m_t[0:DH]
            nb[DH:P, t, 3] = m_t[DH:P]
            coef = -2.0 * ALPHA / SCALE  # -0.0625
            sp[0, t, 0:DH] = coef * m_t[0:DH]
            sp[1, t, DH:P] = coef * m_t[DH:P]
        in_maps.append(
            {
                "xT": np.ascontiguousarray(x[b].T).astype(NPC),
                "wqT": np.ascontiguousarray(Wq[sl, :].T).astype(NPC),
                "wkT": np.ascontiguousarray(Wk[sl, :].T).astype(NPC),
                "wvT": np.ascontiguousarray(Wv[sl, :].T).astype(NPC),
                "woT": np.ascontiguousarray(Wo[:, sl].T).astype(NPC),
                "normblk": nb.astype(NPC),
                "sprime": sp.astype(NPC),
                "maskT": mt.reshape(P, 1, P).astype(NPC),
            }
        )
    return in_maps


def kernel(x, Wq, Wk, Wv, Wo, timelike_mask, attn_mask, _trace=False):
    global _NC
    if _NC is None:
        _NC = _build_program()
    nc = _NC

    x = np.asarray(x, dtype=np.float32)
    Wq, Wk, Wv, Wo = (np.asarray(w, dtype=np.float32) for w in (Wq, Wk, Wv, Wo))
    am = np.asarray(attn_mask, dtype=np.float32).reshape(L, L)
    causal = np.tril(np.ones((L, L), dtype=bool))
    assert np.array_equal(am, np.where(causal, 0.0, -1e9).astype(np.float32)), (
        "kernel hardcodes a causal additive mask"
    )

    in_maps = _host_inputs(x, Wq, Wk, Wv, Wo, timelike_mask)
    res = run_bass_kernel_spmd(
        nc, in_maps, core_ids=list(range(N_CORES)), trace=_trace
    )
    outp = np.stack(
        [
            sum(
                res.results[b * HPC + g]["out"].astype(np.float32)
                for g in range(HPC)
            )
            for b in range(B)
        ]
    )
    kernel.last_results = res
    return outp
